# revision 1
# baseline (speedup 1.0000x reference)
"""GCN block (4x GCNConv w/ symmetric norm + self-loops + ReLU) on 8 TRN2 NeuronCores.

Strategy (dst-sharding, per sharding hint):
  - Nodes are bin-packed (by in-degree) into 128-slot "tiles"; each core owns
    NT tiles. Edges are partitioned by the tile of their *destination*.
  - Per layer, per core:
      agg^T[:, d] = sum_{e: dst=d} norm_e * x[src_e]  +  dinv[d]^2 * x[d]
    computed on the TensorEngine as a sequence of 128-edge "chunk" matmuls
      psum += tokens_chunk^T @ S_chunk         (tokens = gathered x rows)
    where S_chunk[e, d] = (dstlocal_e == d) * norm_e is built by one DVE
    tensor_scalar op per chunk (iota is_equal dstlocal, then mult norm).
    The self-loop term is one extra matmul with a diag(dinv^2) rhs.
    PSUM accumulation performs the segment-sum; the result comes out already
    transposed ([D, nodes]), which feeds the weight matmul directly:
      h = (agg^T)^T @ W  (row-major out),  h += bias,  x' = relu(h)
  - Tokens are fetched with int32-indexed indirect DMA (512B rows) from a
    replicated node-feature buffer that is AllGather'd across the 8 cores
    once per layer (6.5 MB per rank).

Host-side work is limited to index/metadata preprocessing (degrees, norms,
tile assignment, edge bucketing) and data movement (shard/unshard).
"""

import math
import os
import sys

import numpy as np

sys.path.insert(0, "/opt/trn_rl_repo")

NCORES = 8
P = 128          # SBUF partitions == slots per tile == edge-chunk size
D = 128          # feature dim
TG = 4           # tiles per group (4*128 fp32 = one full PSUM bank)

_CACHE = {}


# ----------------------------------------------------------------------------
# Host-side preprocessing (indices / metadata only)
# ----------------------------------------------------------------------------

def _assign_tiles(deg, n_tiles):
    """Balance nodes into n_tiles bins by in-degree, capacity 128 nodes/bin.

    Returns (tile_of[n], slot_of[n]).
    """
    import heapq

    n_nodes = deg.shape[0]
    assert n_tiles * P >= n_nodes
    order = np.argsort(-deg, kind="stable")
    heap = [(0, t) for t in range(n_tiles)]
    heapq.heapify(heap)
    counts = np.zeros(n_tiles, np.int32)
    tile_of = np.empty(n_nodes, np.int32)
    slot_of = np.empty(n_nodes, np.int32)
    for n in order:
        load, t = heapq.heappop(heap)
        tile_of[n] = t
        slot_of[n] = counts[t]
        counts[t] += 1
        if counts[t] < P:
            heapq.heappush(heap, (load + int(deg[n]), t))
    return tile_of, slot_of


def _preprocess(edge_index, n_nodes, nt_per_core):
    """Build all per-core index/metadata arrays."""
    src = np.asarray(edge_index[0], dtype=np.int64)
    dst = np.asarray(edge_index[1], dtype=np.int64)
    n_edges = src.shape[0]
    n_tiles = nt_per_core * NCORES

    indeg = np.bincount(dst, minlength=n_nodes)
    deg = (indeg + 1).astype(np.float32)          # + self loop
    dinv = (np.float32(1.0) / np.sqrt(deg)).astype(np.float32)

    tile_of, slot_of = _assign_tiles(indeg, n_tiles)
    gslot = tile_of.astype(np.int64) * P + slot_of  # node -> global slot

    # --- edge bucketing by dst tile ---
    et = tile_of[dst]                              # edge -> dst tile
    order = np.argsort(et, kind="stable")
    es, ed, et_s = src[order], dst[order], et[order]
    counts = np.bincount(et_s, minlength=n_tiles)
    C = int(math.ceil(counts.max() / P))           # chunks per tile (uniform)
    starts = np.zeros(n_tiles, np.int64)
    starts[1:] = np.cumsum(counts)[:-1]
    rank = np.arange(n_edges, dtype=np.int64) - starts[et_s]
    chunk = rank // P
    eslot = (rank % P).astype(np.int64)
    core_e = et_s // nt_per_core
    col_e = (et_s % nt_per_core) * C + chunk       # column within core arrays

    NTC = nt_per_core * C
    gidx = np.zeros((NCORES, P, NTC), np.int32)
    dstloc = np.full((NCORES, P, NTC), -1.0, np.float32)
    enorm = np.zeros((NCORES, P, NTC), np.float32)
    gidx[core_e, eslot, col_e] = gslot[es].astype(np.int32)
    dstloc[core_e, eslot, col_e] = slot_of[ed].astype(np.float32)
    enorm[core_e, eslot, col_e] = dinv[es] * dinv[ed]

    # --- per-tile diag(dinv^2) column: [NCORES, P, NT] ---
    dinv2 = np.zeros((NCORES, P, nt_per_core), np.float32)
    core_n = tile_of // nt_per_core
    lt_n = tile_of % nt_per_core
    dinv2[core_n, slot_of, lt_n] = dinv * dinv

    return dict(
        gslot=gslot, C=C, gidx=gidx, dstloc=dstloc, enorm=enorm, dinv2=dinv2,
    )


# ----------------------------------------------------------------------------
# Device program
# ----------------------------------------------------------------------------

def _build_program(nt_per_core, C, n_layers):
    import concourse.bass as bass
    import concourse.mybir as mybir
    import concourse.tile as tile
    from concourse import bacc
    from concourse.bass import IndirectOffsetOnAxis

    dt = mybir.dt.float32
    SL = nt_per_core * P                 # slots per core
    NQ = nt_per_core // TG               # tile groups
    NTC = nt_per_core * C
    GC = TG * C                          # chunks per group

    nc = bacc.Bacc(
        "TRN2", target_bir_lowering=False, debug=False, num_devices=NCORES
    )

    x_in = nc.dram_tensor("x_shard", [SL, D], dt, kind="ExternalInput")
    gidx_in = nc.dram_tensor("gidx", [P, NTC], mybir.dt.int32, kind="ExternalInput")
    dl_in = nc.dram_tensor("dstloc", [P, NTC], dt, kind="ExternalInput")
    en_in = nc.dram_tensor("enorm", [P, NTC], dt, kind="ExternalInput")
    d2_in = nc.dram_tensor("dinv2", [P, nt_per_core], dt, kind="ExternalInput")
    si_in = nc.dram_tensor("slotidx", [P, 1], dt, kind="ExternalInput")
    io_in = nc.dram_tensor("iota", [P, P], dt, kind="ExternalInput")
    W_in = nc.dram_tensor("Ws", [n_layers, D, D], dt, kind="ExternalInput")
    bb_in = nc.dram_tensor("bsb", [n_layers, P, TG * D], dt, kind="ExternalInput")
    out_ex = nc.dram_tensor("out", [SL, D], dt, kind="ExternalOutput")

    xsh = [nc.dram_tensor(f"xsh{l}", [SL, D], dt) for l in range(n_layers)]
    xfull = [
        nc.dram_tensor(f"xfull{l}", [NCORES * SL, D], dt, addr_space="Shared")
        for l in range(n_layers)
    ]

    rg = [list(range(NCORES))]

    with tile.TileContext(nc) as tc:
        with (
            tc.tile_pool(name="const", bufs=1) as cp,
            tc.tile_pool(name="tokp", bufs=32) as tokp,
            tc.tile_pool(name="work", bufs=6) as work,
            tc.tile_pool(name="spool", bufs=16) as spool,
            tc.tile_pool(name="psA", bufs=4, space="PSUM") as psA,
            tc.tile_pool(name="psH", bufs=4, space="PSUM") as psH,
        ):
            # ---- resident constants ----
            gidx_sb = cp.tile([P, NTC], mybir.dt.int32)
            nc.sync.dma_start(gidx_sb[:], gidx_in[:])
            dl_sb = cp.tile([P, NTC], dt)
            nc.sync.dma_start(dl_sb[:], dl_in[:])
            en_sb = cp.tile([P, NTC], dt)
            nc.sync.dma_start(en_sb[:], en_in[:])
            d2_sb = cp.tile([P, nt_per_core], dt)
            nc.sync.dma_start(d2_sb[:], d2_in[:])
            si_sb = cp.tile([P, 1], dt)
            nc.sync.dma_start(si_sb[:], si_in[:])
            io_sb = cp.tile([P, P], dt)
            nc.sync.dma_start(io_sb[:], io_in[:])
            W_sb = cp.tile([P, n_layers * D], dt)
            bb_sb = cp.tile([P, n_layers * TG * D], dt)
            for l in range(n_layers):
                nc.sync.dma_start(W_sb[:, l * D:(l + 1) * D], W_in[l])
                nc.sync.dma_start(
                    bb_sb[:, l * TG * D:(l + 1) * TG * D], bb_in[l]
                )

            # ---- stage input shard into an internal buffer, AllGather ----
            nc.sync.dma_start(xsh[0][:], x_in[:])
            nc.gpsimd.collective_compute(
                "AllGather", mybir.AluOpType.bypass, replica_groups=rg,
                ins=[xsh[0][:]], outs=[xfull[0][:]],
            )

            for l in range(n_layers):
                last = l == n_layers - 1
                for q in range(NQ):
                    r0 = q * TG * P                      # first slot row of group
                    # own x rows for the self-loop term
                    xst = work.tile([P, TG * D], dt)
                    nc.sync.dma_start(
                        xst[:].rearrange("p (g d) -> p g d", d=D),
                        xsh[l][r0:r0 + TG * P, :].rearrange(
                            "(g p) d -> p g d", p=P
                        ),
                    )
                    psumA = psA.tile([P, TG * D], dt)
                    for j in range(TG):
                        t = q * TG + j
                        oslice = psumA[:, j * D:(j + 1) * D]
                        for c in range(C):
                            col = t * C + c
                            tok = tokp.tile([P, D], dt)
                            nc.gpsimd.indirect_dma_start(
                                out=tok[:],
                                out_offset=None,
                                in_=xfull[l][:],
                                in_offset=IndirectOffsetOnAxis(
                                    ap=gidx_sb[:, col:col + 1], axis=0
                                ),
                            )
                            S = spool.tile([P, P], dt)
                            nc.vector.tensor_scalar(
                                S[:], io_sb[:],
                                dl_sb[:, col:col + 1],
                                en_sb[:, col:col + 1],
                                op0=mybir.AluOpType.is_equal,
                                op1=mybir.AluOpType.mult,
                            )
                            nc.tensor.matmul(
                                oslice, tok[:], S[:],
                                start=(c == 0), stop=False,
                            )
                        dg = spool.tile([P, P], dt)
                        nc.vector.tensor_scalar(
                            dg[:], io_sb[:], si_sb[:],
                            d2_sb[:, t:t + 1],
                            op0=mybir.AluOpType.is_equal,
                            op1=mybir.AluOpType.mult,
                        )
                        nc.tensor.matmul(
                            oslice, xst[:, j * D:(j + 1) * D], dg[:],
                            start=False, stop=True,
                        )
                    # aggT (PSUM) -> SBUF
                    aggT = work.tile([P, TG * D], dt)
                    nc.scalar.copy(aggT[:], psumA[:])
                    # h = agg @ W  (row-major out)
                    psumH = psH.tile([P, TG * D], dt)
                    for j in range(TG):
                        nc.tensor.matmul(
                            psumH[:, j * D:(j + 1) * D],
                            aggT[:, j * D:(j + 1) * D],
                            W_sb[:, l * D:(l + 1) * D],
                            start=True, stop=True,
                        )
                    # + bias
                    hb = work.tile([P, TG * D], dt)
                    nc.vector.tensor_tensor(
                        hb[:], psumH[:],
                        bb_sb[:, l * TG * D:(l + 1) * TG * D],
                        op=mybir.AluOpType.add,
                    )
                    # relu -> rows
                    xo = work.tile([P, TG * D], dt)
                    nc.scalar.activation(
                        xo[:], hb[:], mybir.ActivationFunctionType.Relu
                    )
                    dst_dram = out_ex if last else xsh[l + 1]
                    nc.sync.dma_start(
                        dst_dram[r0:r0 + TG * P, :].rearrange(
                            "(g p) d -> p g d", p=P
                        ),
                        xo[:].rearrange("p (g d) -> p g d", d=D),
                    )
                if not last:
                    nc.gpsimd.collective_compute(
                        "AllGather", mybir.AluOpType.bypass, replica_groups=rg,
                        ins=[xsh[l + 1][:]], outs=[xfull[l + 1][:]],
                    )

    nc.compile()
    return nc


# ----------------------------------------------------------------------------
# Driver
# ----------------------------------------------------------------------------

def _make_in_maps(x, Ws, bs, pre, nt_per_core):
    n_layers = Ws.shape[0]
    SL = nt_per_core * P
    x = np.asarray(x, np.float32)
    n_nodes = x.shape[0]

    xslots = np.zeros((NCORES * SL, D), np.float32)
    xslots[pre["gslot"]] = x
    xshards = xslots.reshape(NCORES, SL, D)

    slotidx = np.arange(P, dtype=np.float32).reshape(P, 1)
    iota = np.broadcast_to(
        np.arange(P, dtype=np.float32), (P, P)
    ).copy()
    bsb = np.tile(
        np.broadcast_to(
            np.asarray(bs, np.float32)[:, None, :], (n_layers, P, D)
        ),
        (1, 1, TG),
    ).copy()
    Ws_f = np.asarray(Ws, np.float32)

    in_maps = []
    for c in range(NCORES):
        in_maps.append({
            "x_shard": xshards[c],
            "gidx": pre["gidx"][c],
            "dstloc": pre["dstloc"][c],
            "enorm": pre["enorm"][c],
            "dinv2": pre["dinv2"][c],
            "slotidx": slotidx,
            "iota": iota,
            "Ws": Ws_f,
            "bsb": bsb,
        })
    return in_maps


def _ensure_axon_trace_hooks():
    """This image's trn_rl_repo lacks ``antenv.axon_hooks`` (the NTFF
    profile hook shim) — synthesize it and register the ctypes hook from
    trn_agent_boot so ``run_bass_kernel_spmd(trace=True)`` can profile."""
    import types

    if "antenv.axon_hooks" not in sys.modules:
        mod = types.ModuleType("antenv.axon_hooks")
        mod._hook = None
        mod.set_axon_ntff_profile_hook = lambda h: setattr(mod, "_hook", h)
        mod.get_axon_ntff_profile_hook = lambda: mod._hook
        sys.modules["antenv.axon_hooks"] = mod
        try:
            import antenv

            antenv.axon_hooks = mod
        except Exception:
            pass
    mod = sys.modules["antenv.axon_hooks"]
    if mod.get_axon_ntff_profile_hook() is None:
        try:
            from trn_agent_boot.trn_boot import _ntff_profile_via_ctypes

            mod.set_axon_ntff_profile_hook(
                _ntff_profile_via_ctypes("/opt/axon/libaxon_pjrt.so")
            )
        except Exception as e:
            print(f"ntff hook install failed: {e}", file=sys.stderr)
    # artifact upload needs a fish bucket; keep profiles local instead.
    from concourse import bass_utils

    bass_utils.upload_artifacts = lambda tmpdir: tmpdir


def _run(x, Ws, bs, edge_index, mode="hw", trace=False, nt_per_core=104):
    n_nodes = x.shape[0]
    n_layers = Ws.shape[0]
    assert nt_per_core % TG == 0
    assert nt_per_core * P * NCORES >= n_nodes

    pre = _preprocess(edge_index, n_nodes, nt_per_core)
    C = pre["C"]

    key = (nt_per_core, C, n_layers)
    if key not in _CACHE:
        _CACHE[key] = _build_program(nt_per_core, C, n_layers)
    nc = _CACHE[key]

    in_maps = _make_in_maps(x, Ws, bs, pre, nt_per_core)

    if mode == "sim":
        from concourse.bass_interp import MultiCoreSim

        sim = MultiCoreSim(nc, num_cores=NCORES, num_workers=1, trace=False)
        cores = [sim.cores[i] for i in range(NCORES)]
        for c, cs in enumerate(cores):
            for name, arr in in_maps[c].items():
                cs.tensor(name)[:] = arr
        sim.simulate(check_with_hw=False)
        outs = [np.array(cs.tensor("out")) for cs in cores]
        res = None
    else:
        from concourse.bass_utils import run_bass_kernel_spmd

        if trace:
            _ensure_axon_trace_hooks()
        res = run_bass_kernel_spmd(
            nc, in_maps, core_ids=list(range(NCORES)), trace=trace
        )
        outs = [res.results[c]["out"] for c in range(NCORES)]

    full = np.concatenate(outs, axis=0)[pre["gslot"]]
    return np.ascontiguousarray(full, dtype=np.float32), res


def kernel(x, Ws, bs, edge_index):
    mode = os.environ.get("GCN_KERNEL_MODE", "hw")
    trace = os.environ.get("GCN_KERNEL_TRACE", "0") == "1"
    out, _ = _run(
        np.asarray(x), np.asarray(Ws), np.asarray(bs), np.asarray(edge_index),
        mode=mode, trace=trace,
    )
    return out


# ----------------------------------------------------------------------------
# Small-scale self-test (simulator)
# ----------------------------------------------------------------------------

def _ref_numpy(x, Ws, bs, edge_index):
    n = x.shape[0]
    src = np.concatenate([edge_index[0], np.arange(n)])
    dst = np.concatenate([edge_index[1], np.arange(n)])
    deg = np.bincount(dst, minlength=n).astype(np.float32)
    dinv = np.where(deg > 0, 1.0 / np.sqrt(deg), 0.0).astype(np.float32)
    norm = (dinv[src] * dinv[dst])[:, None]
    for i in range(Ws.shape[0]):
        h = x @ Ws[i]
        msg = h[src] * norm
        agg = np.zeros_like(x)
        np.add.at(agg, dst, msg)
        x = np.maximum(agg + bs[i], 0.0)
    return x


def _selftest(n_nodes=3000, n_edges=20000, n_layers=2, nt_per_core=4, seed=0):
    rng = np.random.default_rng(seed)
    x = rng.standard_normal((n_nodes, D), dtype=np.float32)
    Ws = (rng.standard_normal((n_layers, D, D)) / math.sqrt(D)).astype(np.float32)
    bs = (0.1 * rng.standard_normal((n_layers, D))).astype(np.float32)
    edge_index = rng.integers(0, n_nodes, size=(2, n_edges), dtype=np.int64)

    exp = _ref_numpy(x, Ws, bs, edge_index)
    got, _ = _run(x, Ws, bs, edge_index, mode="sim", nt_per_core=nt_per_core)
    err = np.abs(got - exp)
    denom = np.abs(exp).max()
    rel = err.max() / denom
    print(f"selftest: max abs err {err.max():.3e}  rel {rel:.3e}  "
          f"(denom {denom:.3f})")
    assert rel < 1e-4, "selftest FAILED"
    print("selftest PASSED")


if __name__ == "__main__":
    if "--selftest" in sys.argv:
        _selftest()



# revision 5
# speedup vs baseline: 2.2243x; 2.2243x over previous
"""GCN block (4x GCNConv w/ symmetric norm + self-loops + ReLU) on 8 TRN2 NeuronCores.

Strategy (dst-sharding, per sharding hint):
  - Nodes are bin-packed (by in-degree) into 128-slot "tiles"; each core owns
    NT=104 tiles. Edges are partitioned by the tile of their *destination*.
  - Per layer, per core, per 4-tile group:
      agg^T[:, d] = sum_{e: dst=d} norm_e * x[src_e]   (+ self loop)
    computed on the TensorEngine as 128-edge "chunk" matmuls
      psum += tokens_chunk^T @ S_chunk         (tokens = gathered x rows)
    where S_chunk[e, d] = (dstlocal_e == d) * norm_e. The self-loop term is
    one extra "diag" chunk per tile (dstlocal=slot, norm=dinv^2) whose tokens
    are the tile's own rows (direct DMA). PSUM accumulation performs the
    segment-sum; the result comes out transposed ([D, nodes]), feeding the
    weight matmul directly:  h = (agg^T)^T @ W,  x' = relu(h + b).
  - Token fetch: node features live in FOUR interleaved "bank" tensors
    (quarter-shards AllGather'd per layer, bf16). Each (group, bank) issues
    ONE dma_gather of 1024 int16 row indices (tile-pure: 2 chunks per
    (tile, bank)), spread over the 4 SWDGE queues. This amortizes the
    gpsimd descriptor-generation cost (~2.3ns/row) that dominates
    per-chunk indirect DMAs (~8.6ns/row), and int16 indices fit because
    each bank has only 26624 rows.
  - Bank AllGathers for layer l+1 fire as soon as the groups covering that
    quarter of the shard have written, overlapping collectives with the
    tail of layer l.
  - bf16 features / tokens / S / weights end-to-end (PSUM fp32): 4x fewer
    PE cycles per matmul, half the AllGather bytes. S matrices for a whole
    group are built by 2 wide DVE tensor_tensor ops with broadcast (step-0)
    access patterns.

Host-side work is limited to index/metadata preprocessing (degrees, norms,
tile assignment, edge bucketing) and data movement (shard/unshard).
"""

import math
import os
import sys

import numpy as np

sys.path.insert(0, "/opt/trn_rl_repo")

NCORES = 8
P = 128          # SBUF partitions == slots per tile == edge-chunk size
D = 128          # feature dim
TG = 4           # tiles per group (4*128 fp32 = one full PSUM bank)
NBANK = 4        # feature banks (int16 gather index limit)
K = 2            # edge chunks per (tile, bank); capacity 256 edges
CE = NBANK * K   # edge chunk columns per tile (8)
NCOL = TG * CE + TG   # chunk columns per group (36: 32 edge + 4 diag)
GIW = TG * K * P // 16  # idx columns per gather (64)

_CACHE = {}


def _bf16():
    import ml_dtypes

    return ml_dtypes.bfloat16


# ----------------------------------------------------------------------------
# Host-side preprocessing (indices / metadata only)
# ----------------------------------------------------------------------------

def _assign_tiles(deg, n_tiles):
    """Balance nodes into n_tiles bins by in-degree, capacity 128 nodes/bin."""
    import heapq

    n_nodes = deg.shape[0]
    assert n_tiles * P >= n_nodes
    order = np.argsort(-deg, kind="stable")
    heap = [(0, t) for t in range(n_tiles)]
    heapq.heapify(heap)
    counts = np.zeros(n_tiles, np.int32)
    tile_of = np.empty(n_nodes, np.int32)
    slot_of = np.empty(n_nodes, np.int32)
    for n in order:
        load, t = heapq.heappop(heap)
        tile_of[n] = t
        slot_of[n] = counts[t]
        counts[t] += 1
        if counts[t] < P:
            heapq.heappush(heap, (load + int(deg[n]), t))
    return tile_of, slot_of


def _preprocess(edge_index, n_nodes, nt):
    bf16 = _bf16()
    src = np.asarray(edge_index[0], dtype=np.int64)
    dst = np.asarray(edge_index[1], dtype=np.int64)
    n_tiles = nt * NCORES
    QT = nt // NBANK            # tiles per quarter (26)
    BR = QT * P                 # bank rows per core (3328)
    NG = nt // TG               # groups per core (26)

    indeg = np.bincount(dst, minlength=n_nodes)
    deg = (indeg + 1).astype(np.float32)
    dinv = (np.float32(1.0) / np.sqrt(deg)).astype(np.float32)

    tile_of, slot_of = _assign_tiles(indeg, n_tiles)
    gslot = tile_of.astype(np.int64) * P + slot_of

    # per-node bank row (position in the interleaved bank tensors)
    s_core = tile_of // nt
    s_lt = tile_of % nt
    s_bank = s_lt // QT
    s_row = s_core * BR + (s_lt % QT) * P + slot_of      # < NCORES*BR = 26624

    # --- edge bucketing by (dst core, group, bank(src), tile-in-group) ---
    d_tile = tile_of[dst]
    d_core = d_tile // nt
    d_lt = d_tile % nt
    d_g = d_lt // TG
    d_j = d_lt % TG
    e_b = s_bank[src]
    key = ((d_core * NG + d_g) * NBANK + e_b) * TG + d_j
    order = np.argsort(key, kind="stable")
    ks = key[order]
    cnt = np.bincount(ks, minlength=NCORES * NG * NBANK * TG)
    if cnt.max() > K * P:
        raise RuntimeError(f"(tile,bank) bucket overflow: {cnt.max()} > {K*P}")
    starts = np.zeros_like(cnt)
    starts[1:] = np.cumsum(cnt)[:-1]
    pos = np.arange(len(order), dtype=np.int64) - starts[ks]

    eo_src, eo_dst = src[order], dst[order]
    e_core = ks // (NG * NBANK * TG)
    rem = ks % (NG * NBANK * TG)
    e_g = rem // (NBANK * TG)
    rem2 = rem % (NBANK * TG)
    e_bk = rem2 // TG
    e_j = rem2 % TG
    e_k = pos // P
    e_slot = pos % P

    # chunk column within group: edge cols [8b + 2j + k], diag cols [32 + j]
    col = e_g * NCOL + CE * e_bk + 2 * e_j + e_k
    # idx position within the (group, bank) gather: relchunk*128 + slot
    gpos = (e_g * NBANK + e_bk) * (TG * K * P) + (2 * e_j + e_k) * P + e_slot

    NCOLS = NG * NCOL
    dl = np.full((NCORES, P, NCOLS), -1.0, np.float32)
    en = np.zeros((NCORES, P, NCOLS), np.float32)
    dl[e_core, e_slot, col] = slot_of[eo_dst].astype(np.float32)
    en[e_core, e_slot, col] = dinv[eo_src] * dinv[eo_dst]

    # gather idx tables, wrapped in 16 partitions and replicated x8
    NIDX = NG * NBANK * TG * K * P          # idx slots per core (106496)
    lin = np.zeros((NCORES, NIDX), np.int16)
    lin[e_core, gpos] = s_row[eo_src].astype(np.int16)
    wr = lin.reshape(NCORES, NIDX // 16, 16)
    gidx16 = np.empty((NCORES, P, NIDX // 16), np.int16)
    for rep in range(8):
        gidx16[:, rep * 16:(rep + 1) * 16, :] = wr.transpose(0, 2, 1)

    # diag columns: gcol = g*NCOL + TG*CE + j  (dl=slot, en=dinv^2, occupied)
    d2 = np.zeros((n_tiles, P), np.float32)
    d2[tile_of, slot_of] = dinv * dinv
    lt_all = np.arange(n_tiles, dtype=np.int64)
    dcol = (lt_all % nt // TG) * NCOL + TG * CE + (lt_all % nt % TG)
    dcore = lt_all // nt
    pp = np.arange(P, dtype=np.int64)
    ii = dcore[:, None].repeat(P, 1)
    jj = pp[None, :].repeat(n_tiles, 0)
    cc = dcol[:, None].repeat(P, 1)
    dl[ii, jj, cc] = pp[None, :].astype(np.float32)
    en[ii, jj, cc] = d2

    return dict(
        gslot=gslot, gidx16=gidx16,
        dl=dl.astype(bf16), en=en.astype(bf16),
    )


# ----------------------------------------------------------------------------
# Device program
# ----------------------------------------------------------------------------

def _build_program(nt, n_layers, has_bias):
    import concourse.mybir as mybir
    import concourse.tile as tile
    from concourse import bacc
    from concourse.library_config import mlp

    f32 = mybir.dt.float32
    bf = mybir.dt.bfloat16
    SL = nt * P                  # slots per core (13312)
    NG = nt // TG                # groups (26)
    QT = nt // NBANK             # tiles per quarter
    BR = QT * P                  # bank rows per core (3328)
    BANK_ROWS = NCORES * BR      # rows per bank tensor (26624)
    NCOLS = NG * NCOL
    NIW = NG * NBANK * GIW       # gidx16 free size (6656)

    nc = bacc.Bacc(
        "TRN2", target_bir_lowering=False, debug=False, num_devices=NCORES,
        num_swdge_queues=4,
    )

    x_in = nc.dram_tensor("x_shard", [SL, D], bf, kind="ExternalInput")
    gi_in = nc.dram_tensor("gidx16", [P, NIW], mybir.dt.int16,
                           kind="ExternalInput")
    dl_in = nc.dram_tensor("dstloc", [P, NCOLS], bf, kind="ExternalInput")
    en_in = nc.dram_tensor("enorm", [P, NCOLS], bf, kind="ExternalInput")
    io_in = nc.dram_tensor("iota", [P, P], bf, kind="ExternalInput")
    W_in = nc.dram_tensor("Ws", [n_layers, D, D], bf, kind="ExternalInput")
    if has_bias:
        bb_in = nc.dram_tensor(
            "bsb", [n_layers, P, TG * D], f32, kind="ExternalInput"
        )
    out_ex = nc.dram_tensor("out", [SL, D], f32, kind="ExternalOutput")

    xsh = [nc.dram_tensor(f"xsh{l}", [SL, D], bf) for l in range(n_layers)]
    xbank = [
        [
            nc.dram_tensor(f"xb{l}_{b}", [BANK_ROWS, D], bf,
                           addr_space="Shared")
            for b in range(NBANK)
        ]
        for l in range(n_layers)
    ]

    rg = [list(range(NCORES))]
    # group index after which quarter b of the shard is fully written
    cc_after = {}
    for b in range(NBANK):
        g = int(math.ceil(BR * (b + 1) / (TG * P))) - 1
        cc_after.setdefault(g, []).append(b)

    def fire_collectives(l):
        for b in range(NBANK):
            nc.gpsimd.collective_compute(
                "AllGather", mybir.AluOpType.bypass, replica_groups=rg,
                ins=[xsh[l][b * BR:(b + 1) * BR, :]],
                outs=[xbank[l][b][:]],
            )

    with tile.TileContext(nc) as tc:
        with (
            tc.tile_pool(name="const", bufs=1) as cp,
            tc.tile_pool(name="tokp", bufs=5) as tokp,
            tc.tile_pool(name="spool", bufs=4) as spool,
            tc.tile_pool(name="work", bufs=6) as work,
            tc.tile_pool(name="psA", bufs=4, space="PSUM") as psA,
            tc.tile_pool(name="psH", bufs=3, space="PSUM") as psH,
        ):
            nc.gpsimd.load_library(mlp)

            gi_sb = cp.tile([P, NIW], mybir.dt.int16)
            nc.sync.dma_start(gi_sb[:], gi_in[:])
            dl_sb = cp.tile([P, NCOLS], bf)
            nc.sync.dma_start(dl_sb[:], dl_in[:])
            en_sb = cp.tile([P, NCOLS], bf)
            nc.sync.dma_start(en_sb[:], en_in[:])
            io_sb = cp.tile([P, P], bf)
            nc.sync.dma_start(io_sb[:], io_in[:])
            W_sb = cp.tile([P, n_layers * D], bf)
            for l in range(n_layers):
                nc.sync.dma_start(W_sb[:, l * D:(l + 1) * D], W_in[l])
            if has_bias:
                bb_sb = cp.tile([P, n_layers * TG * D], f32)
                for l in range(n_layers):
                    nc.sync.dma_start(
                        bb_sb[:, l * TG * D:(l + 1) * TG * D], bb_in[l]
                    )

            nc.sync.dma_start(xsh[0][:], x_in[:])
            fire_collectives(0)

            for l in range(n_layers):
                last = l == n_layers - 1
                for q in range(NG):
                    r0 = q * TG * P
                    c0 = q * NCOL
                    tok = tokp.tile([P, NCOL * D], bf)
                    for b in range(NBANK):
                        nc.gpsimd.dma_gather(
                            out_ap=tok[:, CE * b * D:CE * (b + 1) * D]
                            .rearrange("p (c e) -> p c e", e=D),
                            in_ap=xbank[l][b][:],
                            idxs_ap=gi_sb[
                                :, (q * NBANK + b) * GIW:(q * NBANK + b + 1) * GIW
                            ],
                            num_idxs=TG * K * P,
                            num_idxs_reg=TG * K * P,
                            elem_size=D,
                            queue_num=b,
                        )
                    # diag tokens: own rows, direct DMA
                    nc.sync.dma_start(
                        tok[:, TG * CE * D:NCOL * D].rearrange(
                            "p (g d) -> p g d", d=D
                        ),
                        xsh[l][r0:r0 + TG * P, :].rearrange(
                            "(g p) d -> p g d", p=P
                        ),
                    )
                    # S for all NCOL chunks in two wide DVE ops
                    S = spool.tile([P, NCOL * P], bf)
                    S3 = S[:].rearrange("p (g k) -> p g k", k=P)
                    io_b = io_sb[:].unsqueeze(1).broadcast_to([P, NCOL, P])
                    dl_b = dl_sb[:, c0:c0 + NCOL].unsqueeze(2).broadcast_to(
                        [P, NCOL, P]
                    )
                    en_b = en_sb[:, c0:c0 + NCOL].unsqueeze(2).broadcast_to(
                        [P, NCOL, P]
                    )
                    nc.vector.tensor_tensor(
                        S3, io_b, dl_b, op=mybir.AluOpType.is_equal
                    )
                    nc.vector.tensor_tensor(
                        S3, S3, en_b, op=mybir.AluOpType.mult
                    )
                    # chunk matmuls (accumulate per tile into psumA slice)
                    psumA = psA.tile([P, TG * D], f32)
                    for j in range(TG):
                        cols = [CE * b + 2 * j + k
                                for b in range(NBANK) for k in range(K)]
                        cols.append(TG * CE + j)
                        oslice = psumA[:, j * D:(j + 1) * D]
                        for ci, c in enumerate(cols):
                            nc.tensor.matmul(
                                oslice,
                                tok[:, c * D:(c + 1) * D],
                                S[:, c * P:(c + 1) * P],
                                start=(ci == 0), stop=(ci == len(cols) - 1),
                            )
                    aggT = work.tile([P, TG * D], bf)
                    nc.scalar.copy(aggT[:], psumA[:])
                    psumH = psH.tile([P, TG * D], f32)
                    for j in range(TG):
                        nc.tensor.matmul(
                            psumH[:, j * D:(j + 1) * D],
                            aggT[:, j * D:(j + 1) * D],
                            W_sb[:, l * D:(l + 1) * D],
                            start=True, stop=True,
                        )
                    odt = f32 if last else bf
                    if has_bias:
                        hb = work.tile([P, TG * D], f32)
                        nc.vector.tensor_tensor(
                            hb[:], psumH[:],
                            bb_sb[:, l * TG * D:(l + 1) * TG * D],
                            op=mybir.AluOpType.add,
                        )
                        xo = work.tile([P, TG * D], odt)
                        nc.scalar.activation(
                            xo[:], hb[:], mybir.ActivationFunctionType.Relu
                        )
                    else:
                        xo = work.tile([P, TG * D], odt)
                        nc.scalar.activation(
                            xo[:], psumH[:], mybir.ActivationFunctionType.Relu
                        )
                    dst_dram = out_ex if last else xsh[l + 1]
                    nc.sync.dma_start(
                        dst_dram[r0:r0 + TG * P, :].rearrange(
                            "(g p) d -> p g d", p=P
                        ),
                        xo[:].rearrange("p (g d) -> p g d", d=D),
                    )
                    if not last:
                        for b in cc_after.get(q, []):
                            nc.gpsimd.collective_compute(
                                "AllGather", mybir.AluOpType.bypass,
                                replica_groups=rg,
                                ins=[xsh[l + 1][b * BR:(b + 1) * BR, :]],
                                outs=[xbank[l + 1][b][:]],
                            )

    nc.compile()
    return nc


# ----------------------------------------------------------------------------
# Driver
# ----------------------------------------------------------------------------

def _make_in_maps(x, Ws, bs, pre, nt, has_bias):
    bf16 = _bf16()
    n_layers = Ws.shape[0]
    SL = nt * P
    x = np.asarray(x, np.float32)

    xslots = np.zeros((NCORES * SL, D), bf16)
    xslots[pre["gslot"]] = x.astype(bf16)
    xshards = xslots.reshape(NCORES, SL, D)

    iota = np.broadcast_to(
        np.arange(P, dtype=np.float32), (P, P)
    ).astype(bf16)
    Ws_b = np.asarray(Ws, np.float32).astype(bf16)

    in_maps = []
    for c in range(NCORES):
        m = {
            "x_shard": xshards[c],
            "gidx16": pre["gidx16"][c],
            "dstloc": pre["dl"][c],
            "enorm": pre["en"][c],
            "iota": iota,
            "Ws": Ws_b,
        }
        if has_bias:
            m["bsb"] = np.tile(
                np.broadcast_to(
                    np.asarray(bs, np.float32)[:, None, :], (n_layers, P, D)
                ),
                (1, 1, TG),
            ).copy()
        in_maps.append(m)
    return in_maps


def _ensure_axon_trace_hooks():
    """This image's trn_rl_repo lacks ``antenv.axon_hooks`` (the NTFF
    profile hook shim) — synthesize it and register the ctypes hook from
    trn_agent_boot so ``run_bass_kernel_spmd(trace=True)`` can profile."""
    import types

    if "antenv.axon_hooks" not in sys.modules:
        mod = types.ModuleType("antenv.axon_hooks")
        mod._hook = None
        mod.set_axon_ntff_profile_hook = lambda h: setattr(mod, "_hook", h)
        mod.get_axon_ntff_profile_hook = lambda: mod._hook
        sys.modules["antenv.axon_hooks"] = mod
        try:
            import antenv

            antenv.axon_hooks = mod
        except Exception:
            pass
    mod = sys.modules["antenv.axon_hooks"]
    if mod.get_axon_ntff_profile_hook() is None:
        try:
            from trn_agent_boot.trn_boot import _ntff_profile_via_ctypes

            mod.set_axon_ntff_profile_hook(
                _ntff_profile_via_ctypes("/opt/axon/libaxon_pjrt.so")
            )
        except Exception as e:
            print(f"ntff hook install failed: {e}", file=sys.stderr)
    from concourse import bass_utils

    bass_utils.upload_artifacts = lambda tmpdir: tmpdir


def _run(x, Ws, bs, edge_index, mode="hw", trace=False, nt_per_core=104):
    n_nodes = x.shape[0]
    n_layers = Ws.shape[0]
    nt = nt_per_core
    assert nt % (TG * NBANK) == 0 or nt % TG == 0 and nt % NBANK == 0
    assert nt * P * NCORES >= n_nodes

    has_bias = bool(np.any(np.asarray(bs)))

    pre = _preprocess(edge_index, n_nodes, nt)

    key = (nt, n_layers, has_bias)
    if key not in _CACHE:
        _CACHE[key] = _build_program(nt, n_layers, has_bias)
    nc = _CACHE[key]

    in_maps = _make_in_maps(x, Ws, bs, pre, nt, has_bias)

    if mode == "sim":
        from concourse.bass_interp import MultiCoreSim

        sim = MultiCoreSim(nc, num_cores=NCORES, num_workers=1, trace=False)
        cores = [sim.cores[i] for i in range(NCORES)]
        for c, cs in enumerate(cores):
            for name, arr in in_maps[c].items():
                cs.tensor(name)[:] = arr
        sim.simulate(check_with_hw=False)
        outs = [np.array(cs.tensor("out")) for cs in cores]
        res = None
    else:
        from concourse.bass_utils import run_bass_kernel_spmd

        if trace:
            _ensure_axon_trace_hooks()
        res = run_bass_kernel_spmd(
            nc, in_maps, core_ids=list(range(NCORES)), trace=trace
        )
        outs = [res.results[c]["out"] for c in range(NCORES)]

    full = np.concatenate(outs, axis=0)[pre["gslot"]]
    return np.ascontiguousarray(full, dtype=np.float32), res


def kernel(x, Ws, bs, edge_index):
    mode = os.environ.get("GCN_KERNEL_MODE", "hw")
    trace = os.environ.get("GCN_KERNEL_TRACE", "0") == "1"
    out, _ = _run(
        np.asarray(x), np.asarray(Ws), np.asarray(bs), np.asarray(edge_index),
        mode=mode, trace=trace,
    )
    return out


# ----------------------------------------------------------------------------
# Small-scale self-test (simulator)
# ----------------------------------------------------------------------------

def _ref_numpy(x, Ws, bs, edge_index):
    n = x.shape[0]
    src = np.concatenate([edge_index[0], np.arange(n)])
    dst = np.concatenate([edge_index[1], np.arange(n)])
    deg = np.bincount(dst, minlength=n).astype(np.float32)
    dinv = np.where(deg > 0, 1.0 / np.sqrt(deg), 0.0).astype(np.float32)
    norm = (dinv[src] * dinv[dst])[:, None]
    for i in range(Ws.shape[0]):
        h = x @ Ws[i]
        msg = h[src] * norm
        agg = np.zeros_like(x)
        np.add.at(agg, dst, msg)
        x = np.maximum(agg + bs[i], 0.0)
    return x


def _selftest(n_nodes=6000, n_edges=30000, n_layers=2, nt_per_core=8, seed=0,
              with_bias=True):
    rng = np.random.default_rng(seed)
    x = rng.standard_normal((n_nodes, D), dtype=np.float32)
    Ws = (rng.standard_normal((n_layers, D, D)) / math.sqrt(D)).astype(np.float32)
    bs = (0.1 * rng.standard_normal((n_layers, D))).astype(np.float32)
    if not with_bias:
        bs = np.zeros_like(bs)
    edge_index = rng.integers(0, n_nodes, size=(2, n_edges), dtype=np.int64)

    exp = _ref_numpy(x, Ws, bs, edge_index)
    got, _ = _run(x, Ws, bs, edge_index, mode="sim", nt_per_core=nt_per_core)
    err = np.abs(got - exp)
    denom = np.abs(exp).max()
    rel = err.max() / denom
    frob = np.linalg.norm(got - exp) / np.linalg.norm(exp)
    print(f"selftest: max abs err {err.max():.3e}  rel {rel:.3e}  "
          f"frob {frob:.3e}  (denom {denom:.3f})")
    assert frob < 5e-3, "selftest FAILED"
    print("selftest PASSED")


if __name__ == "__main__":
    if "--selftest" in sys.argv:
        _selftest()
        _selftest(with_bias=False)
